# revision 16
# baseline (speedup 1.0000x reference)
"""Trainium2 Bass kernel for nn_DilationLayerExtSE (morphological dilation,
external structuring element, per-sample/per-channel weights).

    out[b,c,i,j] = max_{di,dj} (xpad[b,c,i+di,j+dj] + weight[b,c,di,dj]) + bias[b,c]

Shapes (hardcoded): x (8,128,128,128) f32, weight (8,128,5,5) f32,
bias (8,128) f32, padding=2, stride=1 -> out (8,128,128,128) f32.

Sharding: data-parallel over B across the 8 NeuronCores (1 sample/core).
Per core: C=128 maps onto the 128 SBUF partitions; each channel's padded
132x132 plane lives in that partition; bias is folded into the 25 SE
weights on the host (max_k(p+w_k)+b == max_k(p+(w_k+b))).

Engine facts measured on this silicon (per-elem, bf16 SBUF):
  - scalar_tensor_tensor (fused add+max): 1.0 ns — only a 1x uop exists.
  - tensor_tensor (max):                  0.54 ns — 2x_1p mode.
  - tensor_scalar (add, +per-part scalar):0.26 ns on DVE — 4x mode.
  - ACT activation (add via bias):        0.84 ns, dtype-independent.
  - Pool/GPSIMD tensor_scalar:            14 ns — useless; and the V3 ISA
    check rejects every tensor-tensor ALU op on Pool, plus DMA accum-max.

So the optimal split: DVE runs the 24 max passes as pure tensor_tensor at
2x on packed bf16 planes, while the 25 `x_win + w_k` tmp planes are
produced 17 on ACT (activation-identity with per-partition bias) and 8 on
DVE itself with 4x tensor_scalar (the k=0 feed seeds the accumulator
directly).  Accumulator and tmp planes are packed at pitch 128 (windows are
read [rows,128]-strided out of the 132-pitch padded plane), so the max
passes stream fully packed and the store DMA is bulk (1 descriptor per
partition per band).
"""

import os
import time

import numpy as np
import ml_dtypes

B, C, H, W = 8, 128, 128, 128
KH = KW = 5
PAD = 2
HP, WP = H + 2 * PAD, W + 2 * PAD  # 132, 132
NK = KH * KW
XLEN = HP * WP + 8

# Row-band sizes: small first band primes the pipeline.
LANES = os.environ.get("KERNEL_LANES", "8,60,60")
# ks whose tmp feed runs on DVE's 4x tensor_scalar (even dj, spread over di);
# k=0 doubles as the accumulator seed.  The other 17 feeds run on ACT.
DVE_FEED = frozenset(
    int(s) for s in os.environ.get("KERNEL_DVEFEED", "0,2,4,10,12,14,20,22").split(",")
)
TMP_BUFS = int(os.environ.get("KERNEL_TMPBUFS", "6"))
NITER = int(os.environ.get("KERNEL_NITER", "0"))

_CACHE: dict = {}

LAST_RUN_SECONDS: float | None = None
LAST_EXEC_TIME_NS: int | None = None


def _parse_bands():
    bands = []
    r0 = 0
    for part in LANES.split(","):
        rows = int(part.lstrip("v"))
        bands.append((r0, rows))
        r0 += rows
    assert r0 == H, f"bands must cover {H} rows, got {r0}"
    return bands


def _build_program():
    from contextlib import ExitStack

    import concourse.bacc as bacc
    import concourse.tile as tile
    from concourse import mybir

    bands = _parse_bands()

    nc = bacc.Bacc("TRN2", target_bir_lowering=False, debug=False)
    bf = mybir.dt.bfloat16
    f32 = mybir.dt.float32
    x = nc.dram_tensor("x", [C, H * W], bf, kind="ExternalInput")
    w = nc.dram_tensor("w", [C, NK], f32, kind="ExternalInput")
    out = nc.dram_tensor("out", [C, H * W], bf, kind="ExternalOutput")

    add = mybir.AluOpType.add
    mx = mybir.AluOpType.max
    ident = mybir.ActivationFunctionType.Identity

    with tile.TileContext(nc) as tc, ExitStack() as ctx:
        const = ctx.enter_context(tc.tile_pool(name="const", bufs=1))
        acc_p = ctx.enter_context(tc.tile_pool(name="acc", bufs=2))
        tmp_p = ctx.enter_context(tc.tile_pool(name="tmp", bufs=TMP_BUFS))

        xpad = const.tile([C, XLEN], bf)
        wb = const.tile([C, NK], f32)

        xp3 = xpad[:, 0 : HP * WP].rearrange("c (h w) -> c h w", w=WP)
        # zero the pad borders + tail (interior is overwritten per band)
        nc.gpsimd.memset(xpad[:, 0 : PAD * WP], 0.0)
        nc.gpsimd.memset(xpad[:, (HP - PAD) * WP : XLEN], 0.0)
        nc.gpsimd.memset(xp3[:, PAD : HP - PAD, 0:PAD], 0.0)
        nc.gpsimd.memset(xp3[:, PAD : HP - PAD, WP - PAD : WP], 0.0)

        nc.sync.dma_start(out=wb[:], in_=w[:, :])

        def body(_iv=None):
            for r0, rows in bands:
                nc.sync.dma_start(
                    out=xp3[:, PAD + r0 : PAD + r0 + rows, PAD : PAD + W],
                    in_=x[:, r0 * W : (r0 + rows) * W].rearrange(
                        "c (h w) -> c h w", w=W
                    ),
                )
            for r0, rows in bands:

                def win3(k, _r0=r0, _rows=rows):
                    di, dj = divmod(k, KW)
                    base = (_r0 + di) * WP + dj
                    return xpad[:, base : base + _rows * WP].rearrange(
                        "c (h w) -> c h w", w=WP
                    )[:, :, 0:W]

                acc = acc_p.tile([C, rows, W], bf, tag="acc")
                accf = acc.rearrange("c h w -> c (h w)")

                def feed(k, dst3):
                    if k in DVE_FEED:
                        nc.vector.tensor_scalar(
                            out=dst3[:, :, :], in0=win3(k),
                            scalar1=wb[:, k : k + 1], scalar2=None, op0=add,
                        )
                    else:
                        nc.scalar.activation(
                            dst3[:, :, :], win3(k), ident,
                            bias=wb[:, k : k + 1], scale=1.0,
                        )

                feed(0, acc)  # seed
                for k in range(1, NK):
                    tmp = tmp_p.tile([C, rows, W], bf, tag="tmp")
                    feed(k, tmp)
                    tmpf = tmp.rearrange("c h w -> c (h w)")
                    nc.vector.tensor_tensor(
                        out=accf[:, :], in0=tmpf[:, :], in1=accf[:, :], op=mx
                    )
                nc.sync.dma_start(
                    out=out[:, r0 * W : (r0 + rows) * W], in_=accf[:, :]
                )

        if NITER > 0:
            with tc.For_i(0, NITER, 1):
                body()
        else:
            body()

    nc.compile()
    return nc


def _get_nc():
    key = (LANES, NITER)
    if key not in _CACHE:
        _CACHE[key] = _build_program()
    return _CACHE[key]


def kernel(x, weight, bias, padding, stride):
    global LAST_RUN_SECONDS, LAST_EXEC_TIME_NS
    from concourse.bass_utils import run_bass_kernel_spmd

    x = np.asarray(x, dtype=np.float32)
    weight = np.asarray(weight, dtype=np.float32)
    bias = np.asarray(bias, dtype=np.float32)
    assert int(padding) == PAD and int(stride) == 1
    assert x.shape == (B, C, H, W) and weight.shape == (B, C, KH, KW)

    nc = _get_nc()
    xb = x.reshape(B, C, H * W).astype(ml_dtypes.bfloat16)
    wb = (weight.reshape(B, C, NK) + bias[:, :, None]).astype(np.float32)
    in_maps = [
        {
            "x": np.ascontiguousarray(xb[i]),
            "w": np.ascontiguousarray(wb[i]),
        }
        for i in range(B)
    ]
    t0 = time.perf_counter()
    res = run_bass_kernel_spmd(nc, in_maps, core_ids=list(range(B)))
    LAST_RUN_SECONDS = time.perf_counter() - t0
    LAST_EXEC_TIME_NS = res.exec_time_ns
    return np.stack(
        [
            np.asarray(res.results[i]["out"])
            .astype(np.float32)
            .reshape(C, H, W)
            for i in range(B)
        ],
        axis=0,
    )


# revision 20
# speedup vs baseline: 1.0331x; 1.0331x over previous
"""Trainium2 Bass kernel for nn_DilationLayerExtSE (morphological dilation,
external structuring element, per-sample/per-channel weights).

    out[b,c,i,j] = max_{di,dj} (xpad[b,c,i+di,j+dj] + weight[b,c,di,dj]) + bias[b,c]

Shapes (hardcoded): x (8,128,128,128) f32, weight (8,128,5,5) f32,
bias (8,128) f32, padding=2, stride=1 -> out (8,128,128,128) f32.

Sharding: data-parallel over B across the 8 NeuronCores (1 sample/core).
Per core: C=128 maps onto the 128 SBUF partitions; each channel's padded
132x132 plane lives in that partition; bias is folded into the 25 SE
weights on the host (max_k(p+w_k)+b == max_k(p+(w_k+b))).

Engine facts measured on this silicon (per-elem, bf16 SBUF):
  - scalar_tensor_tensor (fused add+max): 1.0 ns — only a 1x uop exists.
  - tensor_tensor (max):                  0.54 ns — 2x_1p mode.
  - tensor_scalar (add, +per-part scalar):0.26 ns on DVE — 4x mode.
  - ACT activation (add via bias):        0.84 ns, dtype-independent.
  - Pool/GPSIMD tensor_scalar:            14 ns — useless; and the V3 ISA
    check rejects every tensor-tensor ALU op on Pool, plus DMA accum-max.

So the optimal split: DVE runs the 24 max passes as pure tensor_tensor at
2x on packed bf16 planes, while the 25 `x_win + w_k` tmp planes are
produced 17 on ACT (activation-identity with per-partition bias) and 8 on
DVE itself with 4x tensor_scalar (the k=0 feed seeds the accumulator
directly).  Accumulator and tmp planes are packed at pitch 128 (windows are
read [rows,128]-strided out of the 132-pitch padded plane), so the max
passes stream fully packed and the store DMA is bulk (1 descriptor per
partition per band).
"""

import os
import time

import numpy as np
import ml_dtypes

B, C, H, W = 8, 128, 128, 128
KH = KW = 5
PAD = 2
HP, WP = H + 2 * PAD, W + 2 * PAD  # 132, 132
NK = KH * KW
XLEN = HP * WP + 8

# Row-band sizes: small first band primes the pipeline.
LANES = os.environ.get("KERNEL_LANES", "8,60,60")
# ks whose tmp feed runs on DVE's 4x tensor_scalar (measured offset-
# insensitive), spread every 3rd k so ACT never has to cover more than two
# consecutive feeds; k=0 doubles as the accumulator seed.  The other 17
# feeds run on ACT.
DVE_FEED = frozenset(
    int(s) for s in os.environ.get("KERNEL_DVEFEED", "0,3,6,9,12,15,18,21").split(",")
)
TMP_BUFS = int(os.environ.get("KERNEL_TMPBUFS", "8"))
# per-band input-DMA dispatch queue: s=SP HWDGE, p=Pool SWDGE, a=ACT HWDGE.
# Multi-queue dispatch (s,p,a) wedged the device inside long For_i timing
# loops (SWDGE ring pressure), so the default stays on the SP ring.
INQ = os.environ.get("KERNEL_INQ", "s").split(",")
NITER = int(os.environ.get("KERNEL_NITER", "0"))

_CACHE: dict = {}

LAST_RUN_SECONDS: float | None = None
LAST_EXEC_TIME_NS: int | None = None


def _parse_bands():
    bands = []
    r0 = 0
    for part in LANES.split(","):
        rows = int(part.lstrip("v"))
        bands.append((r0, rows))
        r0 += rows
    assert r0 == H, f"bands must cover {H} rows, got {r0}"
    return bands


def _build_program():
    from contextlib import ExitStack

    import concourse.bacc as bacc
    import concourse.tile as tile
    from concourse import mybir

    bands = _parse_bands()

    nc = bacc.Bacc("TRN2", target_bir_lowering=False, debug=False)
    bf = mybir.dt.bfloat16
    f32 = mybir.dt.float32
    x = nc.dram_tensor("x", [C, H * W], bf, kind="ExternalInput")
    w = nc.dram_tensor("w", [C, NK], f32, kind="ExternalInput")
    out = nc.dram_tensor("out", [C, H * W], bf, kind="ExternalOutput")

    add = mybir.AluOpType.add
    mx = mybir.AluOpType.max
    ident = mybir.ActivationFunctionType.Identity

    with tile.TileContext(nc) as tc, ExitStack() as ctx:
        const = ctx.enter_context(tc.tile_pool(name="const", bufs=1))
        acc_p = ctx.enter_context(tc.tile_pool(name="acc", bufs=2))
        tmp_p = ctx.enter_context(tc.tile_pool(name="tmp", bufs=TMP_BUFS))

        xpad = const.tile([C, XLEN], bf)
        wb = const.tile([C, NK], f32)

        xp3 = xpad[:, 0 : HP * WP].rearrange("c (h w) -> c h w", w=WP)
        # zero the pad borders + tail (interior is overwritten per band)
        nc.gpsimd.memset(xpad[:, 0 : PAD * WP], 0.0)
        nc.gpsimd.memset(xpad[:, (HP - PAD) * WP : XLEN], 0.0)
        nc.gpsimd.memset(xp3[:, PAD : HP - PAD, 0:PAD], 0.0)
        nc.gpsimd.memset(xp3[:, PAD : HP - PAD, WP - PAD : WP], 0.0)

        nc.sync.dma_start(out=wb[:], in_=w[:, :])

        def body(_iv=None):
            for bi, (r0, rows) in enumerate(bands):
                eng = {"s": nc.sync, "p": nc.gpsimd, "a": nc.scalar}[
                    INQ[bi % len(INQ)]
                ]
                eng.dma_start(
                    out=xp3[:, PAD + r0 : PAD + r0 + rows, PAD : PAD + W],
                    in_=x[:, r0 * W : (r0 + rows) * W].rearrange(
                        "c (h w) -> c h w", w=W
                    ),
                )
            for r0, rows in bands:

                def win3(k, _r0=r0, _rows=rows):
                    di, dj = divmod(k, KW)
                    base = (_r0 + di) * WP + dj
                    return xpad[:, base : base + _rows * WP].rearrange(
                        "c (h w) -> c h w", w=WP
                    )[:, :, 0:W]

                acc = acc_p.tile([C, rows, W], bf, tag="acc")
                accf = acc.rearrange("c h w -> c (h w)")

                def feed(k, dst3):
                    if k in DVE_FEED:
                        nc.vector.tensor_scalar(
                            out=dst3[:, :, :], in0=win3(k),
                            scalar1=wb[:, k : k + 1], scalar2=None, op0=add,
                        )
                    else:
                        nc.scalar.activation(
                            dst3[:, :, :], win3(k), ident,
                            bias=wb[:, k : k + 1], scale=1.0,
                        )

                feed(0, acc)  # seed
                for k in range(1, NK):
                    tmp = tmp_p.tile([C, rows, W], bf, tag="tmp")
                    feed(k, tmp)
                    tmpf = tmp.rearrange("c h w -> c (h w)")
                    nc.vector.tensor_tensor(
                        out=accf[:, :], in0=tmpf[:, :], in1=accf[:, :], op=mx
                    )
                nc.sync.dma_start(
                    out=out[:, r0 * W : (r0 + rows) * W], in_=accf[:, :]
                )

        if NITER > 0:
            with tc.For_i(0, NITER, 1):
                body()
        else:
            body()

    nc.compile()
    return nc


def _get_nc():
    key = (LANES, NITER)
    if key not in _CACHE:
        _CACHE[key] = _build_program()
    return _CACHE[key]


def kernel(x, weight, bias, padding, stride):
    global LAST_RUN_SECONDS, LAST_EXEC_TIME_NS
    from concourse.bass_utils import run_bass_kernel_spmd

    x = np.asarray(x, dtype=np.float32)
    weight = np.asarray(weight, dtype=np.float32)
    bias = np.asarray(bias, dtype=np.float32)
    assert int(padding) == PAD and int(stride) == 1
    assert x.shape == (B, C, H, W) and weight.shape == (B, C, KH, KW)

    nc = _get_nc()
    xb = x.reshape(B, C, H * W).astype(ml_dtypes.bfloat16)
    wb = (weight.reshape(B, C, NK) + bias[:, :, None]).astype(np.float32)
    in_maps = [
        {
            "x": np.ascontiguousarray(xb[i]),
            "w": np.ascontiguousarray(wb[i]),
        }
        for i in range(B)
    ]
    t0 = time.perf_counter()
    res = run_bass_kernel_spmd(nc, in_maps, core_ids=list(range(B)))
    LAST_RUN_SECONDS = time.perf_counter() - t0
    LAST_EXEC_TIME_NS = res.exec_time_ns
    return np.stack(
        [
            np.asarray(res.results[i]["out"])
            .astype(np.float32)
            .reshape(C, H, W)
            for i in range(B)
        ],
        axis=0,
    )


# revision 23
# speedup vs baseline: 1.0747x; 1.0403x over previous
"""Trainium2 Bass kernel for nn_DilationLayerExtSE (morphological dilation,
external structuring element, per-sample/per-channel weights).

    out[b,c,i,j] = max_{di,dj} (xpad[b,c,i+di,j+dj] + weight[b,c,di,dj]) + bias[b,c]

Shapes (hardcoded): x (8,128,128,128) f32, weight (8,128,5,5) f32,
bias (8,128) f32, padding=2, stride=1 -> out (8,128,128,128) f32.

Sharding: data-parallel over B across the 8 NeuronCores (1 sample/core).
Per core: C=128 maps onto the 128 SBUF partitions; each channel's padded
132x132 plane lives in that partition; bias is folded into the 25 SE
weights on the host (max_k(p+w_k)+b == max_k(p+(w_k+b))).

Engine facts measured on this silicon (per-elem, bf16 SBUF):
  - scalar_tensor_tensor (fused add+max): 1.0 ns — only a 1x uop exists.
  - tensor_tensor (max):                  0.54 ns — 2x_1p mode.
  - tensor_scalar (add, +per-part scalar):0.26 ns on DVE — 4x mode.
  - ACT activation (add via bias):        0.84 ns, dtype-independent.
  - Pool/GPSIMD tensor_scalar:            14 ns — useless; and the V3 ISA
    check rejects every tensor-tensor ALU op on Pool, plus DMA accum-max.

So the optimal split: DVE runs the 24 max passes as pure tensor_tensor at
2x on packed bf16 planes, while the 25 `x_win + w_k` tmp planes are
produced 17 on ACT (activation-identity with per-partition bias) and 8 on
DVE itself with 4x tensor_scalar (the k=0 feed seeds the accumulator
directly).  Accumulator and tmp planes are packed at pitch 128 (windows are
read [rows,128]-strided out of the 132-pitch padded plane), so the max
passes stream fully packed and the store DMA is bulk (1 descriptor per
partition per band).
"""

import os
import time

import numpy as np
import ml_dtypes

B, C, H, W = 8, 128, 128, 128
KH = KW = 5
PAD = 2
HP, WP = H + 2 * PAD, W + 2 * PAD  # 132, 132
NK = KH * KW
XLEN = HP * WP + 8

# Row-band sizes: small first band primes the pipeline.
LANES = os.environ.get("KERNEL_LANES", "8,60,60")
# ks whose tmp feed runs on DVE's 4x tensor_scalar (measured offset-
# insensitive), spread every 3rd k so ACT never has to cover more than two
# consecutive feeds; k=0 doubles as the accumulator seed.  The other 17
# feeds run on ACT.
DVE_FEED = frozenset(
    int(s) for s in os.environ.get("KERNEL_DVEFEED", "0,3,6,9,12,15,18,21").split(",")
)
TMP_BUFS = int(os.environ.get("KERNEL_TMPBUFS", "8"))
# FLAT132=1: feeds + all but the last max pass stream the full 132-pitch
# rows (4 garbage cols/row, +3.1% elements) so every AP is 1D; only the
# final pass packs to pitch 128.  Wins iff strided 2D APs cost per-row
# bubbles on ACT/DVE.
FLAT132 = int(os.environ.get("KERNEL_FLAT132", "0"))
# per-band input-DMA dispatch queue: s=SP HWDGE, p=Pool SWDGE, a=ACT HWDGE.
# Multi-queue dispatch (s,p,a) wedged the device inside long For_i timing
# loops (SWDGE ring pressure), so the default stays on the SP ring.
INQ = os.environ.get("KERNEL_INQ", "s").split(",")
NITER = int(os.environ.get("KERNEL_NITER", "0"))

_CACHE: dict = {}

LAST_RUN_SECONDS: float | None = None
LAST_EXEC_TIME_NS: int | None = None


def _parse_bands():
    bands = []
    r0 = 0
    for part in LANES.split(","):
        rows = int(part.lstrip("v"))
        bands.append((r0, rows))
        r0 += rows
    assert r0 == H, f"bands must cover {H} rows, got {r0}"
    return bands


def _build_program():
    from contextlib import ExitStack

    import concourse.bacc as bacc
    import concourse.tile as tile
    from concourse import mybir

    bands = _parse_bands()

    nc = bacc.Bacc("TRN2", target_bir_lowering=False, debug=False)
    bf = mybir.dt.bfloat16
    f32 = mybir.dt.float32
    x = nc.dram_tensor("x", [C, H * W], bf, kind="ExternalInput")
    w = nc.dram_tensor("w", [C, NK], f32, kind="ExternalInput")
    out = nc.dram_tensor("out", [C, H * W], bf, kind="ExternalOutput")

    add = mybir.AluOpType.add
    mx = mybir.AluOpType.max
    ident = mybir.ActivationFunctionType.Identity

    with tile.TileContext(nc) as tc, ExitStack() as ctx:
        const = ctx.enter_context(tc.tile_pool(name="const", bufs=1))
        acc_p = ctx.enter_context(tc.tile_pool(name="acc", bufs=2))
        tmp_p = ctx.enter_context(tc.tile_pool(name="tmp", bufs=TMP_BUFS))
        outp_p = (
            ctx.enter_context(tc.tile_pool(name="outp", bufs=2)) if FLAT132 else None
        )

        xpad = const.tile([C, XLEN], bf)
        wb = const.tile([C, NK], f32)

        xp3 = xpad[:, 0 : HP * WP].rearrange("c (h w) -> c h w", w=WP)
        # zero the pad borders + tail (interior is overwritten per band)
        nc.gpsimd.memset(xpad[:, 0 : PAD * WP], 0.0)
        nc.gpsimd.memset(xpad[:, (HP - PAD) * WP : XLEN], 0.0)
        nc.gpsimd.memset(xp3[:, PAD : HP - PAD, 0:PAD], 0.0)
        nc.gpsimd.memset(xp3[:, PAD : HP - PAD, WP - PAD : WP], 0.0)

        nc.sync.dma_start(out=wb[:], in_=w[:, :])

        def body(_iv=None):
            for bi, (r0, rows) in enumerate(bands):
                eng = {"s": nc.sync, "p": nc.gpsimd, "a": nc.scalar}[
                    INQ[bi % len(INQ)]
                ]
                eng.dma_start(
                    out=xp3[:, PAD + r0 : PAD + r0 + rows, PAD : PAD + W],
                    in_=x[:, r0 * W : (r0 + rows) * W].rearrange(
                        "c (h w) -> c h w", w=W
                    ),
                )
            for r0, rows in bands:

                def win3(k, _r0=r0, _rows=rows):
                    di, dj = divmod(k, KW)
                    base = (_r0 + di) * WP + dj
                    return xpad[:, base : base + _rows * WP].rearrange(
                        "c (h w) -> c h w", w=WP
                    )[:, :, 0:W]

                def winf(k, _r0=r0, _L=rows * WP):
                    di, dj = divmod(k, KW)
                    base = (_r0 + di) * WP + dj
                    return xpad[:, base : base + _L]

                if FLAT132:
                    L = rows * WP
                    acc = acc_p.tile([C, L], bf, tag="acc")

                    def feedf(k, dst):
                        if k in DVE_FEED:
                            nc.vector.tensor_scalar(
                                out=dst, in0=winf(k),
                                scalar1=wb[:, k : k + 1], scalar2=None, op0=add,
                            )
                        else:
                            nc.scalar.activation(
                                dst, winf(k), ident,
                                bias=wb[:, k : k + 1], scale=1.0,
                            )

                    feedf(0, acc[:])  # seed
                    for k in range(1, NK - 1):
                        tmp = tmp_p.tile([C, L], bf, tag="tmp")
                        feedf(k, tmp[:])
                        nc.vector.tensor_tensor(
                            out=acc[:], in0=tmp[:], in1=acc[:], op=mx
                        )
                    # final pass packs to pitch 128
                    tmp = tmp_p.tile([C, L], bf, tag="tmp")
                    feedf(NK - 1, tmp[:])
                    tmp3 = tmp.rearrange("c (h w) -> c h w", w=WP)[:, :, 0:W]
                    acc3 = acc.rearrange("c (h w) -> c h w", w=WP)[:, :, 0:W]
                    outp = outp_p.tile([C, rows, W], bf, tag="outp")
                    nc.vector.tensor_tensor(
                        out=outp[:, :, :], in0=tmp3, in1=acc3, op=mx
                    )
                    nc.sync.dma_start(
                        out=out[:, r0 * W : (r0 + rows) * W],
                        in_=outp.rearrange("c h w -> c (h w)")[:, :],
                    )
                    continue

                acc = acc_p.tile([C, rows, W], bf, tag="acc")
                accf = acc.rearrange("c h w -> c (h w)")

                def feed(k, dst3):
                    if k in DVE_FEED:
                        nc.vector.tensor_scalar(
                            out=dst3[:, :, :], in0=win3(k),
                            scalar1=wb[:, k : k + 1], scalar2=None, op0=add,
                        )
                    else:
                        nc.scalar.activation(
                            dst3[:, :, :], win3(k), ident,
                            bias=wb[:, k : k + 1], scale=1.0,
                        )

                feed(0, acc)  # seed
                for k in range(1, NK):
                    tmp = tmp_p.tile([C, rows, W], bf, tag="tmp")
                    feed(k, tmp)
                    tmpf = tmp.rearrange("c h w -> c (h w)")
                    nc.vector.tensor_tensor(
                        out=accf[:, :], in0=tmpf[:, :], in1=accf[:, :], op=mx
                    )
                nc.sync.dma_start(
                    out=out[:, r0 * W : (r0 + rows) * W], in_=accf[:, :]
                )

        if NITER > 0:
            with tc.For_i(0, NITER, 1):
                body()
        else:
            body()

    nc.compile()
    return nc


def _get_nc():
    key = (LANES, NITER)
    if key not in _CACHE:
        _CACHE[key] = _build_program()
    return _CACHE[key]


def kernel(x, weight, bias, padding, stride):
    global LAST_RUN_SECONDS, LAST_EXEC_TIME_NS
    from concourse.bass_utils import run_bass_kernel_spmd

    x = np.asarray(x, dtype=np.float32)
    weight = np.asarray(weight, dtype=np.float32)
    bias = np.asarray(bias, dtype=np.float32)
    assert int(padding) == PAD and int(stride) == 1
    assert x.shape == (B, C, H, W) and weight.shape == (B, C, KH, KW)

    nc = _get_nc()
    xb = x.reshape(B, C, H * W).astype(ml_dtypes.bfloat16)
    wb = (weight.reshape(B, C, NK) + bias[:, :, None]).astype(np.float32)
    in_maps = [
        {
            "x": np.ascontiguousarray(xb[i]),
            "w": np.ascontiguousarray(wb[i]),
        }
        for i in range(B)
    ]
    t0 = time.perf_counter()
    res = run_bass_kernel_spmd(nc, in_maps, core_ids=list(range(B)))
    LAST_RUN_SECONDS = time.perf_counter() - t0
    LAST_EXEC_TIME_NS = res.exec_time_ns
    return np.stack(
        [
            np.asarray(res.results[i]["out"])
            .astype(np.float32)
            .reshape(C, H, W)
            for i in range(B)
        ],
        axis=0,
    )
